# revision 31
# baseline (speedup 1.0000x reference)
"""Trainium2 Bass kernel for nn_Attention_53334903882008 (additive attention).

Reference (per batch b):
  We  = img @ W^T + Wb;  Ue = (hid @ U^T + Ub) broadcast over T
  att = tanh(We + Ue);   e = att @ w + wb
  alpha = softmax_N(e);  phi = sum_n alpha * img      -> [B, T, D]

Sharding: data-parallel over B=8, one batch per NeuronCore; weights
replicated. Per-core dataflow (v2, [btn, h] orientation):
  - U_comb = hid @ U^T + (Wb + Ub) is computed on the HOST (tiny, 34M MACs)
    and uploaded pre-broadcast as ucomb2 [128, H] (stacked twice along n),
    killing the whole on-device U/hT startup phase.
  - x = img[b] ([8192, 1024] fp32) is cast fp32->bf16 during the SWDGE DMA
    load (natural [btn, d] layout, rhs of the phi matmul) and xbar-DMA/PE
    transposed into [d, btn] tiles.
  - We is computed as [btn, h] tiles: lhsT = x^T tile (stationary),
    rhs = W^T [d, h] (moving, FD=512 = full H). 8 accumulating matmuls per
    btn-tile. This puts h on the FREE axis, so:
  - e = w.att contracts h on the free axis via ONE fused DVE
    tensor_tensor_reduce (att*w2 with accumulated add, seeded with w_bias)
    -> e lands per-partition [128, 1] natively: no e-matmuls and no
    exp->partition K=1 transposes on the PE at all.
  - The (Wb+Ub)+U@hid addend is added in-place into PSUM by one DVE op
    (ucomb2 is constant across tiles in t-major order), then ScalarE tanh.
  - Softmax over N=64 runs unnormalized; exp per tile on ScalarE [128, 1];
    the 1/s(t) scale is folded into the final phi PSUM->SBUF eviction.
  - phi accumulates over all 64 btn-tiles into persistent [t, d] PSUM via
    block-diagonal alpha matrices in PARITY-MAJOR t-order (r = 64*(t%2) +
    t//2) so the per-t sums (built by one mask matmul over exp values +
    two K=1 row->column matmuls) line up with PSUM partitions; the output
    DMA un-permutes rows for free via its DRAM access pattern.
  - The Tile sem-assigner globally fences Transpose-mode DMAs against
    Copy-mode DMAs (xbar-hang workaround), so casts/transposes are batched
    in chunk groups to amortize the mode-switch drains. Chunks 0-1 are
    PE-transposed (DMA not warm yet at startup, and it warms the HAM).
  - phi matmuls for tile j are interleaved into the We stream of tile j+3
    (the DVE/ACT chain needs ~2.3us to produce adiag_j), keeping the PE
    dense with no cross-chunk pending machinery.
"""

from contextlib import ExitStack

import numpy as np
import ml_dtypes

import concourse.bacc as bacc
import concourse.tile as tile
from concourse import mybir
from concourse.bass_utils import run_bass_kernel_spmd

B = 8

BF = mybir.dt.bfloat16
F32 = mybir.dt.float32
FP8 = mybir.dt.float8e4
NPBF = ml_dtypes.bfloat16
NPF8 = ml_dtypes.float8_e4m3

T, N, D, H = 128, 64, 1024, 512
BTN = T * N            # 8192
NCH = 8                # chunks over btn
CH = BTN // NCH        # 1024 btn per chunk
JT = CH // 128         # 8 btn-tiles (of 128) per chunk
KT = D // 128          # 8 contraction tiles
NI = BTN // 128        # 64 btn-tiles total
LAG = 3                # phi matmuls trail the We stream by LAG tiles


def build(nc):
    x_d = nc.dram_tensor("x", [BTN, D], F32, kind="ExternalInput").ap()
    wtm_d = nc.dram_tensor("wtm", [128, KT * H], BF, kind="ExternalInput").ap()
    wt8_d = nc.dram_tensor("wt8", [128, 2 * 2 * H], FP8, kind="ExternalInput").ap()
    uc2_d = nc.dram_tensor("ucomb2", [128, H], BF, kind="ExternalInput").ap()
    w2_d = nc.dram_tensor("w2", [128, H], BF, kind="ExternalInput").ap()
    wb_d = nc.dram_tensor("wbcol", [128, 1], F32, kind="ExternalInput").ap()
    bw_d = nc.dram_tensor("base", [128, 191], BF, kind="ExternalInput").ap()
    gm_d = nc.dram_tensor("gatemask", [128, 128], BF, kind="ExternalInput").ap()
    i128_d = nc.dram_tensor("i128", [128, 128], BF, kind="ExternalInput").ap()
    on_d = nc.dram_tensor("ones", [128, 1], BF, kind="ExternalInput").ap()
    phi_d = nc.dram_tensor("phi", [T, D], F32, kind="ExternalOutput").ap()

    with tile.TileContext(nc) as tc, ExitStack() as ctx:
        consts = ctx.enter_context(tc.tile_pool(name="consts", bufs=1))
        xnp = ctx.enter_context(tc.tile_pool(name="xnat", bufs=6))
        xtp = ctx.enter_context(tc.tile_pool(name="xT", bufs=2))
        # DMA-transposed chunks' xT tiles are produced early (fence slot)
        # but consumed last -> own pool so they don't wrap xtp's ring.
        xtd = ctx.enter_context(tc.tile_pool(name="xTd", bufs=2))
        attp = ctx.enter_context(tc.tile_pool(name="att", bufs=3))
        scrp = ctx.enter_context(tc.tile_pool(name="scr", bufs=2))
        adp = ctx.enter_context(tc.tile_pool(name="adiag", bufs=4))
        psm1 = ctx.enter_context(tc.tile_pool(name="psmm1", bufs=5, space="PSUM"))
        pssm = ctx.enter_context(tc.tile_pool(name="pssml", bufs=1, space="PSUM"))
        psph = ctx.enter_context(tc.tile_pool(name="psphi", bufs=1, space="PSUM"))

        # ---- constants (i128 + wtm first: they gate the first PE work) ----
        i128 = consts.tile([128, 128], BF)
        nc.sync.dma_start(out=i128, in_=i128_d)
        wtm = consts.tile([128, KT, H], BF)
        nc.sync.dma_start(out=wtm, in_=wtm_d.rearrange("p (k h) -> p k h", k=KT))
        # fp8 (e4m3) copy of W^T for kt 4-7, pre-scaled x32, in DoubleRow
        # pair layout: wt8[p, pair, i, h] = 32*W[h, (4+2*pair+i)*128+p]
        wt8 = consts.tile([128, 2, 2, H], FP8)
        nc.sync.dma_start(
            out=wt8, in_=wt8_d.rearrange("p (a b h) -> p a b h", a=2, b=2)
        )
        ucomb2 = consts.tile([128, H], BF)
        nc.sync.dma_start(out=ucomb2, in_=uc2_d)
        w2 = consts.tile([128, H], BF)
        nc.sync.dma_start(out=w2, in_=w2_d)
        base = consts.tile([128, 191], BF)
        nc.sync.dma_start(out=base, in_=bw_d)
        gatemask = consts.tile([128, 128], BF)
        nc.sync.dma_start(out=gatemask, in_=gm_d)
        onec = consts.tile([128, 1], BF)
        nc.sync.dma_start(out=onec, in_=on_d)
        wbcol = consts.tile([128, 1], F32)
        nc.sync.dma_start(out=wbcol, in_=wb_d)

        # ---- persistent softmax state ----
        e_all = consts.tile([128, NI], F32)     # e values (sans wb), col = btn-tile
        eexp = consts.tile([128, NI], F32)      # exp(e + wb)
        eexp2 = consts.tile([128, 2, NI], BF)   # gated bf16 repeat for s-MM
        ps_phi0 = psph.tile([T, 512], F32, tag="phi0")
        ps_phi1 = psph.tile([T, 512], F32, tag="phi1")
        ps_phi = [ps_phi0, ps_phi1]

        # ---- chunk pipeline ----
        def emit_cast(c, quarters=1):
            xn = xnp.tile([128, JT, D], BF, tag="xn")
            src = (
                x_d.rearrange("(a p) d -> a p d", p=128)[c * JT : (c + 1) * JT]
                .rearrange("a p d -> p a d")
            )
            q = JT // quarters
            for i in range(quarters):
                nc.gpsimd.dma_start(
                    out=xn[:, i * q : (i + 1) * q, :], in_=src[:, i * q : (i + 1) * q, :]
                )  # cast f32 -> bf16
            return xn

        def emit_transpose(xn):
            xT = xtd.tile([128, JT, KT, 128], BF, tag="xtd")
            nc.sync.dma_start(
                out=xT.rearrange("p j k c -> p (j k) c"),
                in_=xn.rearrange("p j d -> p (j d)"),
                transpose=True,
            )
            return xT

        def emit_one_tr(xn, j):
            # transpose tile j on the TensorE instead of the xbar DMA:
            # the sem-assigner's global transpose-vs-copy DMA fence makes
            # the DMA wire (casts + transposes, ~176us serial) the kernel
            # bottleneck otherwise. ~56ns/tile warm on the PE.
            ps_t = psm1.tile([128, KT, 128], BF, tag="mm1")
            for kt in range(KT):
                nc.tensor.transpose(
                    ps_t[:, kt, :],
                    xn[:, j, kt * 128 : (kt + 1) * 128],
                    i128,
                )
            return ps_t

        def emit_evict(ps_t, xT, xT8, j):
            # kt 0-3 evicted bf16; kt 4-7 evicted as e4m3 scaled 1/32 (the
            # matching W tiles are pre-scaled x32) for DoubleRow matmuls.
            # Split across DVE and ACT to balance the load.
            if j % 2 == 0:
                nc.vector.tensor_copy(xT[:, j], ps_t[:, 0:4, :])
                nc.scalar.activation(
                    xT8[:, j].rearrange("p a b c -> p (a b) c"),
                    ps_t[:, 4:8, :],
                    mybir.ActivationFunctionType.Copy,
                    scale=1.0 / 32.0,
                )
            else:
                nc.scalar.activation(
                    xT[:, j], ps_t[:, 0:4, :], mybir.ActivationFunctionType.Copy
                )
                nc.vector.tensor_scalar_mul(
                    xT8[:, j].rearrange("p a b c -> p (a b) c"),
                    ps_t[:, 4:8, :],
                    1.0 / 32.0,
                )

        # Deferred per-tile tails: (gj, xn) -> emit adiag + 2 phi matmuls.
        def emit_phi(gj, xn):
            j = gj % JT
            adiag = adp.tile([128, 128], BF, tag="ad")
            nc.vector.tensor_scalar_mul(
                adiag, base[:, 63 - gj : 191 - gj], eexp[:, gj : gj + 1]
            )
            for dh in range(2):
                nc.tensor.matmul(
                    ps_phi[dh],
                    lhsT=adiag,
                    rhs=xn[:, j, dh * 512 : (dh + 1) * 512],
                    start=(gj == 0),
                    stop=(gj == NI - 1),
                )

        def emit_tile(c, j, xn, xT, xT8):
            # accumulating We matmuls: out [btn, h], lhsT = x^T (stationary).
            # PE-transposed chunks run kt 0-3 in bf16 and kt 4-7 as two
            # DoubleRow fp8 matmuls (1.8x streaming for that half).
            gj = c * JT + j
            ps = psm1.tile([128, H], F32, tag="mm1")
            nkt = 4 if xT8 is not None else KT
            for kt in range(nkt):
                nc.tensor.matmul(
                    ps,
                    lhsT=xT[:, j, kt, :],
                    rhs=wtm[:, kt, :],
                    start=(kt == 0),
                    stop=(xT8 is None and kt == KT - 1),
                )
            if xT8 is not None:
                for pair in range(2):
                    nc.tensor.matmul(
                        ps,
                        lhsT=xT8[:, j, pair],
                        rhs=wt8[:, pair],
                        start=False,
                        stop=(pair == 1),
                        perf_mode=mybir.MatmulPerfMode.DoubleRow,
                    )
            # + U addend, then tanh -> att (bf16)
            pre = scrp.tile([128, H], F32, tag="pre")
            nc.vector.tensor_tensor(
                out=pre, in0=ps, in1=ucomb2, op=mybir.AluOpType.add
            )
            att = attp.tile([128, H], BF, tag="att")
            nc.scalar.activation(att, pre, mybir.ActivationFunctionType.Tanh)
            # e = sum_h att*w along the free axis (GpSimd mult + DVE reduce;
            # tensor_tensor_reduce crashes TRN2 HW, so two standard ops),
            # with w_bias folded into the exp's per-partition ACT bias.
            scr = scrp.tile([128, H], BF, tag="scr")
            nc.gpsimd.tensor_tensor(
                out=scr, in0=att, in1=w2, op=mybir.AluOpType.mult
            )
            nc.vector.reduce_sum(
                out=e_all[:, gj : gj + 1], in_=scr, axis=mybir.AxisListType.X
            )
            nc.scalar.activation(
                eexp[:, gj : gj + 1],
                e_all[:, gj : gj + 1],
                mybir.ActivationFunctionType.Exp,
                bias=wbcol,
            )

        # PE-transposed chunks 0..NPE-1 consumed first (their casts are
        # front-loaded on the wire); DMA-transposed chunks NPE..7 last,
        # their [cast][transpose] pairs emitted at the ends of chunks 1/2
        # (early enough on the wire, late enough not to block the gpsimd
        # queue on an xn-slot wait).
        NPE = 6
        phi_q = []  # tiles whose phi matmuls are pending
        xns, xts, xt8s = {}, {}, {}
        for c in range(NPE):
            xns[c] = emit_cast(c, quarters=8 if c == 0 else 1)
        for c in range(NCH):
            if c < NPE:
                # transposes j0-j4 up front, j5-j7 staggered into the loop;
                # evictions run two tiles ahead of the We matmuls so the
                # 5-slot PSUM ring (tr tiles + We accumulators) never
                # blocks on a not-yet-emitted reader.
                ps_ts = {j: emit_one_tr(xns[c], j) for j in range(5)}
                xT_c = xtp.tile([128, JT, 4, 128], BF, tag="xt")
                xT8_c = xtp.tile([128, JT, 2, 2, 128], FP8, tag="xt8")
                xts[c], xt8s[c] = xT_c, xT8_c
                emit_evict(ps_ts.pop(0), xts[c], xt8s[c], 0)
                emit_evict(ps_ts.pop(1), xts[c], xt8s[c], 1)
            for j in range(JT):
                if c < NPE and j < 3:
                    ps_ts[j + 5] = emit_one_tr(xns[c], j + 5)
                    emit_evict(ps_ts.pop(2 * j + 2), xts[c], xt8s[c], 2 * j + 2)
                    emit_evict(ps_ts.pop(2 * j + 3), xts[c], xt8s[c], 2 * j + 3)
                # emit the trailing phi work FIRST so its DVE adiag op
                # sits ahead of tile j's add/mult on the DVE queue (its
                # deps are LAG tiles old and already satisfied).
                while len(phi_q) >= LAG:
                    emit_phi(*phi_q.pop(0))
                emit_tile(c, j, xns[c], xts[c], xt8s.get(c))
                phi_q.append((c * JT + j, xns[c]))
            if c == 1:
                xns[6] = emit_cast(6)
                xts[6] = emit_transpose(xns[6])
            elif c == 2:
                xns[7] = emit_cast(7)
                xts[7] = emit_transpose(xns[7])
        for item in phi_q:
            emit_phi(*item)

        # ---- finalize: s(t) sums, phi = ps_phi * (1/s_t) ----
        # eexp2[p, k, i] = exp(e)[p, i] * gate_k[p] (gate: k==p//64), so one
        # K=128 matmul against ones gives ps_scol[r] = s at parity-major row
        # r directly: r<64 -> sum over p<64 of col r; r>=64 -> p>=64 half.
        for k in range(2):
            nc.vector.tensor_tensor(
                out=eexp2[:, k, :], in0=eexp,
                in1=gatemask[:, k * 64 : k * 64 + NI],
                op=mybir.AluOpType.mult,
            )
        ps_scol = pssm.tile([128, 1], F32, tag="sml")
        nc.tensor.matmul(
            ps_scol, lhsT=eexp2.rearrange("p a b -> p (a b)"), rhs=onec,
            start=True, stop=True,
        )
        recip = consts.tile([128, 1], F32)
        nc.vector.reciprocal(recip, ps_scol)
        phi_sb = consts.tile([T, D], F32)
        for dh in range(2):
            nc.vector.tensor_scalar_mul(
                phi_sb[:, dh * 512 : (dh + 1) * 512], ps_phi[dh], recip
            )
        # rows are parity-major (r = 64*(t%2) + t//2); un-permute via two
        # strided DRAM writes (even t rows, then odd t rows).
        phi_v = phi_d.rearrange("(i p) d -> i p d", p=2)
        nc.sync.dma_start(out=phi_v[:, 0, :], in_=phi_sb[0:64, :])
        nc.sync.dma_start(out=phi_v[:, 1, :], in_=phi_sb[64:128, :])

    return nc


def prep_consts(W_weight, w_weight, w_bias):
    # W^T tiles as the MOVING operand: wtm[p, kt*H + h] = W[h, kt*128+p]
    WT = W_weight.T.astype(np.float32)  # [D, H]
    wtm = np.ascontiguousarray(
        WT.reshape(KT, 128, H).transpose(1, 0, 2).reshape(128, KT * H)
    ).astype(NPBF)
    # parity-major block-diag indicator: col 63 for p<64, col 127 for p>=64
    base = np.zeros((128, 191), np.float32)
    for p in range(128):
        base[p, 63 + 64 * (p // 64)] = 1.0
    # gatemask[:, 0:64]: rows p<64 ones (even-t gate); [:, 64:128]: p>=64
    gm = np.zeros((128, 128), np.float32)
    gm[:64, 0:64] = 1.0
    gm[64:, 64:128] = 1.0
    # wt8[p, pair, i, h] = 32*W[h, (4+2*pair+i)*128+p] in e4m3 (DoubleRow
    # pair layout for kt 4-7; the x^T side is scaled 1/32 at eviction)
    wt8 = np.empty((128, 2, 2, H), np.float32)
    for pair in range(2):
        for i in range(2):
            kt = 4 + 2 * pair + i
            wt8[:, pair, i, :] = 32.0 * WT[kt * 128 : (kt + 1) * 128, :]
    return {
        "wtm": wtm,
        "wt8": np.ascontiguousarray(wt8.reshape(128, 2 * 2 * H)).astype(NPF8),
        "w2": np.ascontiguousarray(
            np.broadcast_to(w_weight[0][None, :], (128, H))
        ).astype(NPBF),
        "wbcol": np.full((128, 1), float(w_bias[0]), np.float32),
        "base": base.astype(NPBF),
        "gatemask": gm.astype(NPBF),
        "i128": np.eye(128, dtype=np.float32).astype(NPBF),
        "ones": np.ones((128, 1), NPBF),
    }


_NC_CACHE = {}


def make_nc(num_devices=B):
    if num_devices not in _NC_CACHE:
        nc = bacc.Bacc(
            "TRN2", target_bir_lowering=False, debug=False, num_devices=num_devices
        )
        build(nc)
        nc.compile()
        _NC_CACHE[num_devices] = nc
    return _NC_CACHE[num_devices]


def prep_in_maps(img_features, hidden_state, U_weight, W_bias, U_bias, consts):
    # U_comb = hid_b @ U^T + (Wb + Ub), host-side (34M MACs/batch), stacked
    # twice along n to match t-major btn tiles (partition p -> n = p%64).
    in_maps = []
    for b in range(B):
        uc = (
            hidden_state[:, b, :].astype(np.float32) @ U_weight.T.astype(np.float32)
            + W_bias + U_bias
        )  # [N, H]
        uc2 = np.ascontiguousarray(np.concatenate([uc, uc], axis=0)).astype(NPBF)
        in_maps.append(
            {
                "x": np.ascontiguousarray(
                    img_features[b].reshape(BTN, D), dtype=np.float32
                ),
                "ucomb2": uc2,
                **consts,
            }
        )
    return in_maps


def run(inputs, trace=False, tmpdir=None):
    """Run the SPMD kernel; returns (phi [B,T,D] fp32, BassKernelResults)."""
    inputs = {k: np.asarray(v) for k, v in inputs.items()}
    consts = prep_consts(inputs["W_weight"], inputs["w_weight"], inputs["w_bias"])
    in_maps = prep_in_maps(
        inputs["img_features"], inputs["hidden_state"], inputs["U_weight"],
        inputs["W_bias"], inputs["U_bias"], consts,
    )
    nc = make_nc(B)
    last_err = None
    for attempt in range(3):
        try:
            res = run_bass_kernel_spmd(
                nc, in_maps, core_ids=list(range(B)), trace=trace, tmpdir=tmpdir
            )
            break
        except Exception as e:  # transient NRT_EXEC_UNIT_UNRECOVERABLE etc.
            last_err = e
            if "UNRECOVERABLE" not in str(e) and "UNAVAILABLE" not in str(e):
                raise
    else:
        raise last_err
    phi = np.stack([res.results[b]["phi"] for b in range(B)]).astype(np.float32)
    return phi, res


def kernel(**inputs) -> np.ndarray:
    phi, _ = run(inputs, trace=False)
    return phi


# revision 32
# speedup vs baseline: 1.3066x; 1.3066x over previous
"""Trainium2 Bass kernel for nn_Attention_53334903882008 (additive attention).

Reference (per batch b):
  We  = img @ W^T + Wb;  Ue = (hid @ U^T + Ub) broadcast over T
  att = tanh(We + Ue);   e = att @ w + wb
  alpha = softmax_N(e);  phi = sum_n alpha * img      -> [B, T, D]

Sharding: data-parallel over B=8, one batch per NeuronCore; weights
replicated. Per-core dataflow (v2, [btn, h] orientation):
  - U_comb = hid @ U^T + (Wb + Ub) is computed on the HOST (tiny, 34M MACs)
    and uploaded pre-broadcast as ucomb2 [128, H] (stacked twice along n),
    killing the whole on-device U/hT startup phase.
  - x = img[b] ([8192, 1024] fp32) is cast fp32->bf16 during the SWDGE DMA
    load (natural [btn, d] layout, rhs of the phi matmul) and xbar-DMA/PE
    transposed into [d, btn] tiles.
  - We is computed as [btn, h] tiles: lhsT = x^T tile (stationary),
    rhs = W^T [d, h] (moving, FD=512 = full H). 8 accumulating matmuls per
    btn-tile. This puts h on the FREE axis, so:
  - e = w.att contracts h on the free axis via ONE fused DVE
    tensor_tensor_reduce (att*w2 with accumulated add, seeded with w_bias)
    -> e lands per-partition [128, 1] natively: no e-matmuls and no
    exp->partition K=1 transposes on the PE at all.
  - The (Wb+Ub)+U@hid addend is added in-place into PSUM by one DVE op
    (ucomb2 is constant across tiles in t-major order), then ScalarE tanh.
  - Softmax over N=64 runs unnormalized; exp per tile on ScalarE [128, 1];
    the 1/s(t) scale is folded into the final phi PSUM->SBUF eviction.
  - phi accumulates over all 64 btn-tiles into persistent [t, d] PSUM via
    block-diagonal alpha matrices in PARITY-MAJOR t-order (r = 64*(t%2) +
    t//2) so the per-t sums (built by one mask matmul over exp values +
    two K=1 row->column matmuls) line up with PSUM partitions; the output
    DMA un-permutes rows for free via its DRAM access pattern.
  - The Tile sem-assigner globally fences Transpose-mode DMAs against
    Copy-mode DMAs (xbar-hang workaround), so casts/transposes are batched
    in chunk groups to amortize the mode-switch drains. Chunks 0-1 are
    PE-transposed (DMA not warm yet at startup, and it warms the HAM).
  - phi matmuls for tile j are interleaved into the We stream of tile j+3
    (the DVE/ACT chain needs ~2.3us to produce adiag_j), keeping the PE
    dense with no cross-chunk pending machinery.
"""

from contextlib import ExitStack

import numpy as np
import ml_dtypes

import concourse.bacc as bacc
import concourse.tile as tile
from concourse import mybir
from concourse.bass_utils import run_bass_kernel_spmd

B = 8

BF = mybir.dt.bfloat16
F32 = mybir.dt.float32
FP8 = mybir.dt.float8e4
NPBF = ml_dtypes.bfloat16
NPF8 = ml_dtypes.float8_e4m3

T, N, D, H = 128, 64, 1024, 512
BTN = T * N            # 8192
NCH = 8                # chunks over btn
CH = BTN // NCH        # 1024 btn per chunk
JT = CH // 128         # 8 btn-tiles (of 128) per chunk
KT = D // 128          # 8 contraction tiles
NI = BTN // 128        # 64 btn-tiles total
LAG = 3                # phi matmuls trail the We stream by LAG tiles


def build(nc):
    x_d = nc.dram_tensor("x", [BTN, D], F32, kind="ExternalInput").ap()
    wtm_d = nc.dram_tensor("wtm", [128, KT * H], BF, kind="ExternalInput").ap()
    wt8_d = nc.dram_tensor("wt8", [128, 2 * 2 * H], FP8, kind="ExternalInput").ap()
    uc2_d = nc.dram_tensor("ucomb2", [128, H], BF, kind="ExternalInput").ap()
    w2_d = nc.dram_tensor("w2", [128, H], BF, kind="ExternalInput").ap()
    wb_d = nc.dram_tensor("wbcol", [128, 1], F32, kind="ExternalInput").ap()
    bw_d = nc.dram_tensor("base", [128, 191], BF, kind="ExternalInput").ap()
    gm_d = nc.dram_tensor("gatemask", [128, 128], BF, kind="ExternalInput").ap()
    i128_d = nc.dram_tensor("i128", [128, 128], BF, kind="ExternalInput").ap()
    on_d = nc.dram_tensor("ones", [128, 1], BF, kind="ExternalInput").ap()
    phi_d = nc.dram_tensor("phi", [T, D], F32, kind="ExternalOutput").ap()

    with tile.TileContext(nc) as tc, ExitStack() as ctx:
        consts = ctx.enter_context(tc.tile_pool(name="consts", bufs=1))
        xnp = ctx.enter_context(tc.tile_pool(name="xnat", bufs=6))
        xtp = ctx.enter_context(tc.tile_pool(name="xT", bufs=2))
        # DMA-transposed chunks' xT tiles are produced early (fence slot)
        # but consumed last -> own pool so they don't wrap xtp's ring.
        xtd = ctx.enter_context(tc.tile_pool(name="xTd", bufs=2))
        attp = ctx.enter_context(tc.tile_pool(name="att", bufs=3))
        scrp = ctx.enter_context(tc.tile_pool(name="scr", bufs=2))
        adp = ctx.enter_context(tc.tile_pool(name="adiag", bufs=4))
        psm1 = ctx.enter_context(tc.tile_pool(name="psmm1", bufs=5, space="PSUM"))
        pssm = ctx.enter_context(tc.tile_pool(name="pssml", bufs=1, space="PSUM"))
        psph = ctx.enter_context(tc.tile_pool(name="psphi", bufs=1, space="PSUM"))

        # ---- constants (i128 + wtm first: they gate the first PE work) ----
        i128 = consts.tile([128, 128], BF)
        nc.sync.dma_start(out=i128, in_=i128_d)
        wtm = consts.tile([128, KT, H], BF)
        nc.sync.dma_start(out=wtm, in_=wtm_d.rearrange("p (k h) -> p k h", k=KT))
        # fp8 (e4m3) copy of W^T for kt 4-7, pre-scaled x32, in DoubleRow
        # pair layout: wt8[p, pair, i, h] = 32*W[h, (4+2*pair+i)*128+p]
        wt8 = consts.tile([128, 2, 2, H], FP8)
        nc.sync.dma_start(
            out=wt8, in_=wt8_d.rearrange("p (a b h) -> p a b h", a=2, b=2)
        )
        ucomb2 = consts.tile([128, H], BF)
        nc.sync.dma_start(out=ucomb2, in_=uc2_d)
        w2 = consts.tile([128, H], BF)
        nc.sync.dma_start(out=w2, in_=w2_d)
        base = consts.tile([128, 191], BF)
        nc.sync.dma_start(out=base, in_=bw_d)
        gatemask = consts.tile([128, 128], BF)
        nc.sync.dma_start(out=gatemask, in_=gm_d)
        onec = consts.tile([128, 1], BF)
        nc.sync.dma_start(out=onec, in_=on_d)
        wbcol = consts.tile([128, 1], F32)
        nc.sync.dma_start(out=wbcol, in_=wb_d)

        # ---- persistent softmax state ----
        e_all = consts.tile([128, NI], F32)     # e values (sans wb), col = btn-tile
        eexp = consts.tile([128, NI], F32)      # exp(e + wb)
        eexp2 = consts.tile([128, 2, NI], BF)   # gated bf16 repeat for s-MM
        ps_phi0 = psph.tile([T, 512], F32, tag="phi0")
        ps_phi1 = psph.tile([T, 512], F32, tag="phi1")
        ps_phi = [ps_phi0, ps_phi1]

        # ---- chunk pipeline ----
        def emit_cast(c, quarters=1):
            xn = xnp.tile([128, JT, D], BF, tag="xn")
            src = (
                x_d.rearrange("(a p) d -> a p d", p=128)[c * JT : (c + 1) * JT]
                .rearrange("a p d -> p a d")
            )
            q = JT // quarters
            for i in range(quarters):
                nc.gpsimd.dma_start(
                    out=xn[:, i * q : (i + 1) * q, :], in_=src[:, i * q : (i + 1) * q, :]
                )  # cast f32 -> bf16
            return xn

        def emit_transpose(xn):
            xT = xtd.tile([128, JT, KT, 128], BF, tag="xtd")
            nc.sync.dma_start(
                out=xT.rearrange("p j k c -> p (j k) c"),
                in_=xn.rearrange("p j d -> p (j d)"),
                transpose=True,
            )
            return xT

        def emit_one_tr(xn, j):
            # transpose tile j on the TensorE instead of the xbar DMA:
            # the sem-assigner's global transpose-vs-copy DMA fence makes
            # the DMA wire (casts + transposes, ~176us serial) the kernel
            # bottleneck otherwise. ~56ns/tile warm on the PE.
            ps_t = psm1.tile([128, KT, 128], BF, tag="mm1")
            for kt in range(KT):
                nc.tensor.transpose(
                    ps_t[:, kt, :],
                    xn[:, j, kt * 128 : (kt + 1) * 128],
                    i128,
                )
            return ps_t

        def emit_evict(ps_t, xT, xT8, j):
            # kt 0-3 evicted bf16; kt 4-7 evicted as e4m3 scaled 1/32 (the
            # matching W tiles are pre-scaled x32) for DoubleRow matmuls.
            # Split across DVE and ACT to balance the load.
            if j % 2 == 0:
                nc.vector.tensor_copy(xT[:, j], ps_t[:, 0:4, :])
                nc.scalar.activation(
                    xT8[:, j].rearrange("p a b c -> p (a b) c"),
                    ps_t[:, 4:8, :],
                    mybir.ActivationFunctionType.Copy,
                    scale=1.0 / 32.0,
                )
            else:
                nc.scalar.activation(
                    xT[:, j], ps_t[:, 0:4, :], mybir.ActivationFunctionType.Copy
                )
                nc.vector.tensor_scalar_mul(
                    xT8[:, j].rearrange("p a b c -> p (a b) c"),
                    ps_t[:, 4:8, :],
                    1.0 / 32.0,
                )

        # Deferred per-tile tails: (gj, xn) -> emit adiag + 2 phi matmuls.
        def emit_phi(gj, xn):
            j = gj % JT
            adiag = adp.tile([128, 128], BF, tag="ad")
            nc.vector.tensor_scalar_mul(
                adiag, base[:, 63 - gj : 191 - gj], eexp[:, gj : gj + 1]
            )
            for dh in range(2):
                nc.tensor.matmul(
                    ps_phi[dh],
                    lhsT=adiag,
                    rhs=xn[:, j, dh * 512 : (dh + 1) * 512],
                    start=(gj == 0),
                    stop=(gj == NI - 1),
                )

        def emit_tile(c, j, xn, xT, xT8):
            # accumulating We matmuls: out [btn, h], lhsT = x^T (stationary).
            # PE-transposed chunks run kt 0-3 in bf16 and kt 4-7 as two
            # DoubleRow fp8 matmuls (1.8x streaming for that half).
            gj = c * JT + j
            ps = psm1.tile([128, H], F32, tag="mm1")
            nkt = 4 if xT8 is not None else KT
            for kt in range(nkt):
                nc.tensor.matmul(
                    ps,
                    lhsT=xT[:, j, kt, :],
                    rhs=wtm[:, kt, :],
                    start=(kt == 0),
                    stop=(xT8 is None and kt == KT - 1),
                )
            if xT8 is not None:
                for pair in range(2):
                    nc.tensor.matmul(
                        ps,
                        lhsT=xT8[:, j, pair],
                        rhs=wt8[:, pair],
                        start=False,
                        stop=(pair == 1),
                        perf_mode=mybir.MatmulPerfMode.DoubleRow,
                    )
            # + U addend, then tanh -> att (bf16)
            pre = scrp.tile([128, H], F32, tag="pre")
            nc.vector.tensor_tensor(
                out=pre, in0=ps, in1=ucomb2, op=mybir.AluOpType.add
            )
            att = attp.tile([128, H], BF, tag="att")
            nc.scalar.activation(att, pre, mybir.ActivationFunctionType.Tanh)
            # e = sum_h att*w along the free axis (DVE mult + reduce;
            # tensor_tensor_reduce crashes TRN2 HW and GpSimd is ~4x
            # slower per op, so two standard DVE ops), with w_bias folded
            # into the exp's per-partition ACT bias.
            scr = scrp.tile([128, H], BF, tag="scr")
            nc.vector.tensor_tensor(
                out=scr, in0=att, in1=w2, op=mybir.AluOpType.mult
            )
            nc.vector.reduce_sum(
                out=e_all[:, gj : gj + 1], in_=scr, axis=mybir.AxisListType.X
            )
            nc.scalar.activation(
                eexp[:, gj : gj + 1],
                e_all[:, gj : gj + 1],
                mybir.ActivationFunctionType.Exp,
                bias=wbcol,
            )

        # PE-transposed chunks 0..NPE-1 consumed first (their casts are
        # front-loaded on the wire); DMA-transposed chunks NPE..7 last,
        # their [cast][transpose] pairs emitted at the ends of chunks 1/2
        # (early enough on the wire, late enough not to block the gpsimd
        # queue on an xn-slot wait).
        NPE = 6
        phi_q = []  # tiles whose phi matmuls are pending
        xns, xts, xt8s = {}, {}, {}
        for c in range(NPE):
            xns[c] = emit_cast(c, quarters=8 if c == 0 else 1)
        for c in range(NCH):
            if c < NPE:
                # transposes j0-j4 up front, j5-j7 staggered into the loop;
                # evictions run two tiles ahead of the We matmuls so the
                # 5-slot PSUM ring (tr tiles + We accumulators) never
                # blocks on a not-yet-emitted reader.
                ps_ts = {j: emit_one_tr(xns[c], j) for j in range(5)}
                xT_c = xtp.tile([128, JT, 4, 128], BF, tag="xt")
                xT8_c = xtp.tile([128, JT, 2, 2, 128], FP8, tag="xt8")
                xts[c], xt8s[c] = xT_c, xT8_c
                emit_evict(ps_ts.pop(0), xts[c], xt8s[c], 0)
                emit_evict(ps_ts.pop(1), xts[c], xt8s[c], 1)
            for j in range(JT):
                if c < NPE and j < 3:
                    ps_ts[j + 5] = emit_one_tr(xns[c], j + 5)
                    emit_evict(ps_ts.pop(2 * j + 2), xts[c], xt8s[c], 2 * j + 2)
                    emit_evict(ps_ts.pop(2 * j + 3), xts[c], xt8s[c], 2 * j + 3)
                # emit the trailing phi work FIRST so its DVE adiag op
                # sits ahead of tile j's add/mult on the DVE queue (its
                # deps are LAG tiles old and already satisfied).
                while len(phi_q) >= LAG:
                    emit_phi(*phi_q.pop(0))
                emit_tile(c, j, xns[c], xts[c], xt8s.get(c))
                phi_q.append((c * JT + j, xns[c]))
            if c == 1:
                xns[6] = emit_cast(6)
                xts[6] = emit_transpose(xns[6])
            elif c == 2:
                xns[7] = emit_cast(7)
                xts[7] = emit_transpose(xns[7])
        for item in phi_q:
            emit_phi(*item)

        # ---- finalize: s(t) sums, phi = ps_phi * (1/s_t) ----
        # eexp2[p, k, i] = exp(e)[p, i] * gate_k[p] (gate: k==p//64), so one
        # K=128 matmul against ones gives ps_scol[r] = s at parity-major row
        # r directly: r<64 -> sum over p<64 of col r; r>=64 -> p>=64 half.
        for k in range(2):
            nc.vector.tensor_tensor(
                out=eexp2[:, k, :], in0=eexp,
                in1=gatemask[:, k * 64 : k * 64 + NI],
                op=mybir.AluOpType.mult,
            )
        ps_scol = pssm.tile([128, 1], F32, tag="sml")
        nc.tensor.matmul(
            ps_scol, lhsT=eexp2.rearrange("p a b -> p (a b)"), rhs=onec,
            start=True, stop=True,
        )
        recip = consts.tile([128, 1], F32)
        nc.vector.reciprocal(recip, ps_scol)
        phi_sb = consts.tile([T, D], F32)
        for dh in range(2):
            nc.vector.tensor_scalar_mul(
                phi_sb[:, dh * 512 : (dh + 1) * 512], ps_phi[dh], recip
            )
        # rows are parity-major (r = 64*(t%2) + t//2); un-permute via two
        # strided DRAM writes (even t rows, then odd t rows).
        phi_v = phi_d.rearrange("(i p) d -> i p d", p=2)
        nc.sync.dma_start(out=phi_v[:, 0, :], in_=phi_sb[0:64, :])
        nc.sync.dma_start(out=phi_v[:, 1, :], in_=phi_sb[64:128, :])

    return nc


def prep_consts(W_weight, w_weight, w_bias):
    # W^T tiles as the MOVING operand: wtm[p, kt*H + h] = W[h, kt*128+p]
    WT = W_weight.T.astype(np.float32)  # [D, H]
    wtm = np.ascontiguousarray(
        WT.reshape(KT, 128, H).transpose(1, 0, 2).reshape(128, KT * H)
    ).astype(NPBF)
    # parity-major block-diag indicator: col 63 for p<64, col 127 for p>=64
    base = np.zeros((128, 191), np.float32)
    for p in range(128):
        base[p, 63 + 64 * (p // 64)] = 1.0
    # gatemask[:, 0:64]: rows p<64 ones (even-t gate); [:, 64:128]: p>=64
    gm = np.zeros((128, 128), np.float32)
    gm[:64, 0:64] = 1.0
    gm[64:, 64:128] = 1.0
    # wt8[p, pair, i, h] = 32*W[h, (4+2*pair+i)*128+p] in e4m3 (DoubleRow
    # pair layout for kt 4-7; the x^T side is scaled 1/32 at eviction)
    wt8 = np.empty((128, 2, 2, H), np.float32)
    for pair in range(2):
        for i in range(2):
            kt = 4 + 2 * pair + i
            wt8[:, pair, i, :] = 32.0 * WT[kt * 128 : (kt + 1) * 128, :]
    return {
        "wtm": wtm,
        "wt8": np.ascontiguousarray(wt8.reshape(128, 2 * 2 * H)).astype(NPF8),
        "w2": np.ascontiguousarray(
            np.broadcast_to(w_weight[0][None, :], (128, H))
        ).astype(NPBF),
        "wbcol": np.full((128, 1), float(w_bias[0]), np.float32),
        "base": base.astype(NPBF),
        "gatemask": gm.astype(NPBF),
        "i128": np.eye(128, dtype=np.float32).astype(NPBF),
        "ones": np.ones((128, 1), NPBF),
    }


_NC_CACHE = {}


def make_nc(num_devices=B):
    if num_devices not in _NC_CACHE:
        nc = bacc.Bacc(
            "TRN2", target_bir_lowering=False, debug=False, num_devices=num_devices
        )
        build(nc)
        nc.compile()
        _NC_CACHE[num_devices] = nc
    return _NC_CACHE[num_devices]


def prep_in_maps(img_features, hidden_state, U_weight, W_bias, U_bias, consts):
    # U_comb = hid_b @ U^T + (Wb + Ub), host-side (34M MACs/batch), stacked
    # twice along n to match t-major btn tiles (partition p -> n = p%64).
    in_maps = []
    for b in range(B):
        uc = (
            hidden_state[:, b, :].astype(np.float32) @ U_weight.T.astype(np.float32)
            + W_bias + U_bias
        )  # [N, H]
        uc2 = np.ascontiguousarray(np.concatenate([uc, uc], axis=0)).astype(NPBF)
        in_maps.append(
            {
                "x": np.ascontiguousarray(
                    img_features[b].reshape(BTN, D), dtype=np.float32
                ),
                "ucomb2": uc2,
                **consts,
            }
        )
    return in_maps


def run(inputs, trace=False, tmpdir=None):
    """Run the SPMD kernel; returns (phi [B,T,D] fp32, BassKernelResults)."""
    inputs = {k: np.asarray(v) for k, v in inputs.items()}
    consts = prep_consts(inputs["W_weight"], inputs["w_weight"], inputs["w_bias"])
    in_maps = prep_in_maps(
        inputs["img_features"], inputs["hidden_state"], inputs["U_weight"],
        inputs["W_bias"], inputs["U_bias"], consts,
    )
    nc = make_nc(B)
    last_err = None
    for attempt in range(3):
        try:
            res = run_bass_kernel_spmd(
                nc, in_maps, core_ids=list(range(B)), trace=trace, tmpdir=tmpdir
            )
            break
        except Exception as e:  # transient NRT_EXEC_UNIT_UNRECOVERABLE etc.
            last_err = e
            if "UNRECOVERABLE" not in str(e) and "UNAVAILABLE" not in str(e):
                raise
    else:
        raise last_err
    phi = np.stack([res.results[b]["phi"] for b in range(B)]).astype(np.float32)
    return phi, res


def kernel(**inputs) -> np.ndarray:
    phi, _ = run(inputs, trace=False)
    return phi
